# revision 1
# baseline (speedup 1.0000x reference)
"""Trainium2 Bass kernel v2 for DeepSet MLP (embedding-lookup-sum + MLP).

Math: u[b] = sum_j W_phi[x[b,j]] + N*b_phi; y = relu(relu(u@W1+b1)@W2+b2)@W3+b3.

Scheme (per core, 512 rows, data-parallel over 8 cores):
  - Class split c = 32*hi + lo. Host sends pre-transposed split index
    tensors xloQ/xhiQ (bf16) in quad layout [j%128, (blk4, i4, bq32, r4)].
  - One-hots in quad-interleaved layout hq[p, (i, bq, lo, r)] /
    gq[p, (i, bq, h, r)] so that (a) the per-quad matmul stationary
    slice (lo,r)=128 cols is contiguous (FWL fast weight load) and
    (b) writes are 4-elem contiguous runs (TS mode) or fully dense
    (TT broadcast mode).
  - Count matmuls: per 4-row quad, 4 j-chunk matmuls accumulate in
    PSUM: out[(lo,r), (h,r')] holds complete counts on r==r' diagonal.
  - Evacuation: whole PSUM banks (8 quads) copied to SBUF bf16 in one
    contiguous ACT copy (counts are small ints: bf16-exact).
  - Projection u = counts @ W_phi via diagonal-absorbing matmuls: for
    each (r', h) a K=128 matmul whose stationary has W_phi rows (32h+lo)
    embedded at partitions (lo, r') and zeros elsewhere.
  - MLP in bf16 with mean-centered activations: the host folds the
    large constant offsets (N*b_phi path) into per-layer f32 biases and
    subtracts per-layer means so every bf16 tensor is small.
"""

import os
import numpy as np
from contextlib import ExitStack

import concourse.bass as bass
import concourse.bacc as bacc
import concourse.tile as tile
import concourse.mybir as mybir
from concourse.bass_utils import run_bass_kernel_spmd

B, N, C, PHI = 4096, 512, 512, 128
H1, H2 = 512, 256
NCORES = 8
BS = B // NCORES          # 512 rows per core
NI = 4                    # j chunks of 128
NBLK = 4                  # row blocks, uneven: tail block small
QS = [40, 40, 32, 16]     # quads per block (sum 128, each %8==0)
QSTART = [0, 40, 80, 112]
BQB = 32                  # (legacy name, unused in loops)
LO, HI = 32, 16

F32 = mybir.dt.float32
BF16 = mybir.dt.bfloat16
ALU = mybir.AluOpType
AF = mybir.ActivationFunctionType

ONEHOT_MODE = os.environ.get("K_ONEHOT", "ts")  # "ts" or "tt"


def build_program():
    nc = bacc.Bacc("TRN2", target_bir_lowering=False, debug=False,
                   num_devices=NCORES)

    xlo = nc.dram_tensor("xlo", [128, NBLK * NI * BQB * 4], BF16,
                         kind="ExternalInput")
    xhi = nc.dram_tensor("xhi", [128, NBLK * NI * BQB * 4], BF16,
                         kind="ExternalInput")
    iolo = nc.dram_tensor("iolo", [128, LO], BF16, kind="ExternalInput")
    iohi = nc.dram_tensor("iohi", [128, HI], BF16, kind="ExternalInput")
    wselz = nc.dram_tensor("wselz", [128, 4 * HI * PHI], BF16,
                           kind="ExternalInput")
    w1 = nc.dram_tensor("w1", [PHI, H1], BF16, kind="ExternalInput")
    b1p = nc.dram_tensor("b1p", [128, 4], F32, kind="ExternalInput")
    c1n = nc.dram_tensor("c1n", [128, 4], F32, kind="ExternalInput")
    w2 = nc.dram_tensor("w2", [128, 4 * 2 * 128], BF16, kind="ExternalInput")
    b2p = nc.dram_tensor("b2p", [128, 2], F32, kind="ExternalInput")
    c2n = nc.dram_tensor("c2n", [128, 2], F32, kind="ExternalInput")
    w3 = nc.dram_tensor("w3", [128, 2], BF16, kind="ExternalInput")
    off0 = nc.dram_tensor("off0", [128, 1], F32, kind="ExternalInput")
    b3p = nc.dram_tensor("b3p", [1, 1], F32, kind="ExternalInput")
    out = nc.dram_tensor("out", [1, BS], F32, kind="ExternalOutput")

    with tile.TileContext(nc) as tc:
        with ExitStack() as ctx:
            _emit(ctx, tc, nc, xlo, xhi, iolo, iohi, wselz, w1, b1p, c1n,
                  w2, b2p, c2n, w3, off0, b3p, out)
    nc.compile()
    return nc


def _emit(ctx, tc, nc, xlo, xhi, iolo, iohi, wselz, w1, b1p, c1n, w2, b2p,
          c2n, w3, off0, b3p, out):
    consts = ctx.enter_context(tc.tile_pool(name="consts", bufs=1))
    ohp = ctx.enter_context(tc.tile_pool(name="ohp", bufs=2))
    mlp = ctx.enter_context(tc.tile_pool(name="mlp", bufs=1))
    xin = ctx.enter_context(tc.tile_pool(name="xin", bufs=1))
    ps_cnt = ctx.enter_context(tc.tile_pool(name="ps_cnt", bufs=2,
                                            space="PSUM"))
    ps_u = ctx.enter_context(tc.tile_pool(name="ps_u", bufs=1, space="PSUM"))
    ps_mlp = ctx.enter_context(tc.tile_pool(name="ps_mlp", bufs=2,
                                            space="PSUM"))
    ps_w2 = ctx.enter_context(tc.tile_pool(name="ps_w2", bufs=1,
                                           space="PSUM"))
    ps_y = ctx.enter_context(tc.tile_pool(name="ps_y", bufs=1, space="PSUM"))

    # Index tensors first: the one-hot pipeline depends only on these, so
    # they must not queue behind the big constant DMAs.
    xls = [xin.tile([128, NI * QS[b] * 4], BF16, name=f"xl{b}")
           for b in range(NBLK)]
    xhs = [xin.tile([128, NI * QS[b] * 4], BF16, name=f"xh{b}")
           for b in range(NBLK)]
    for b in range(NBLK):
        o0, o1 = NI * QSTART[b] * 4, NI * (QSTART[b] + QS[b]) * 4
        nc.sync.dma_start(xls[b][:], xlo.ap()[:, o0:o1])
        nc.sync.dma_start(xhs[b][:], xhi.ap()[:, o0:o1])
    iolo_sb = consts.tile([128, LO], BF16)
    nc.sync.dma_start(iolo_sb[:], iolo.ap())
    iohi_sb = consts.tile([128, HI], BF16)
    nc.sync.dma_start(iohi_sb[:], iohi.ap())
    wselz_sb = consts.tile([128, 4 * HI * PHI], BF16)
    nc.sync.dma_start(wselz_sb[:], wselz.ap())
    w1sb = consts.tile([128, H1], BF16)
    nc.sync.dma_start(w1sb[:], w1.ap())
    b1psb = consts.tile([128, 4], F32)
    nc.sync.dma_start(b1psb[:], b1p.ap())
    c1nsb = consts.tile([128, 4], F32)
    nc.sync.dma_start(c1nsb[:], c1n.ap())
    w2sb = consts.tile([128, 4 * 2 * 128], BF16)
    nc.sync.dma_start(w2sb[:], w2.ap())
    b2psb = consts.tile([128, 2], F32)
    nc.sync.dma_start(b2psb[:], b2p.ap())
    c2nsb = consts.tile([128, 2], F32)
    nc.sync.dma_start(c2nsb[:], c2n.ap())
    w3sb = consts.tile([128, 2], BF16)
    nc.sync.dma_start(w3sb[:], w3.ap())
    off0sb = consts.tile([128, 1], F32)
    nc.sync.dma_start(off0sb[:], off0.ap())
    b3psb = consts.tile([1, 1], F32)
    nc.sync.dma_start(b3psb[:], b3p.ap())

    # counts: [128=(lo32,r4), (h16, r'4, bq128)] bf16 — h/r'-major so the
    # projection's moving-operand columns are contiguous
    fvq = mlp.tile([128, 128 * HI * 4], BF16)
    fvh = fvq[:].rearrange("p (h r q) -> p h r q", h=HI, r=4)

    for blk in range(NBLK):
        QB = QS[blk]
        xl = xls[blk][:]
        xh = xhs[blk][:]

        hq = ohp.tile([128, NI * 40 * LO * 4], BF16, tag="hq")
        gq = ohp.tile([128, NI * 40 * HI * 4], BF16, tag="gq")
        hqv = hq[:].rearrange("p (i q l r) -> p i q l r", i=NI, q=40, l=LO)
        hqv = hqv[:, :, :QB, :, :]
        gqv = gq[:].rearrange("p (i q l r) -> p i q l r", i=NI, q=40, l=HI)
        gqv = gqv[:, :, :QB, :, :]
        xlv = xl.rearrange("p (i q r) -> p i q r", i=NI, q=QB)
        xhv = xh.rearrange("p (i q r) -> p i q r", i=NI, q=QB)
        del xl, xh

        if ONEHOT_MODE == "ts":
            for lo in range(LO):
                nc.vector.tensor_scalar(out=hqv[:, :, :, lo, :], in0=xlv,
                                        scalar1=lo, scalar2=None,
                                        op0=ALU.is_equal)
            for h in range(HI):
                nc.vector.tensor_scalar(out=gqv[:, :, :, h, :], in0=xhv,
                                        scalar1=h, scalar2=None,
                                        op0=ALU.is_equal)
        else:
            in0l = xlv.unsqueeze(3).broadcast_to([128, NI, QB, LO, 4])
            in1l = iolo_sb[:].unsqueeze(1).unsqueeze(1).unsqueeze(4) \
                .broadcast_to([128, NI, QB, LO, 4])
            nc.vector.tensor_tensor(out=hqv, in0=in0l, in1=in1l,
                                    op=ALU.is_equal)
            in0h = xhv.unsqueeze(3).broadcast_to([128, NI, QB, HI, 4])
            in1h = iohi_sb[:, :HI].unsqueeze(1).unsqueeze(1).unsqueeze(4) \
                .broadcast_to([128, NI, QB, HI, 4])
            nc.vector.tensor_tensor(out=gqv, in0=in0h, in1=in1h,
                                    op=ALU.is_equal)

        hqm = hq[:].rearrange("p (i q c) -> p i q c", i=NI, q=40)
        gqm = gq[:].rearrange("p (i q c) -> p i q c", i=NI, q=40)

        for t in range(QB // 8):
            pt = ps_cnt.tile([128, 512], F32, tag="cnt")
            for q8 in range(8):
                qb = t * 8 + q8
                for i in range(NI):
                    nc.tensor.matmul(
                        pt[:, q8 * 64:(q8 + 1) * 64],
                        hqm[:, i, qb, :],
                        gqm[:, i, qb, :],
                        start=(i == 0), stop=(i == NI - 1))
            # transposing evacuation: psum (q8, h, r') -> fvq (h, r', bq);
            # dst writes are 8-elem (16B) contiguous runs
            bq0 = QSTART[blk] + t * 8
            dst = fvh[:, :, :, bq0:bq0 + 8]
            srcap = pt[:].rearrange("p (q h r) -> p h r q", q=8, h=HI)
            nc.scalar.copy(dst, srcap)

    # ---- projection with diagonal-absorbing zero-padded stationaries ----
    # u columns in (r, k) order: each r-chain writes a contiguous 128-col
    # PSUM region (stride-4 PSUM writes quadruple the matmul drain time).
    # The b-order is restored by the output DMA's read pattern.
    pu = ps_u.tile([128, BS], F32)
    for r in range(4):
        for h in range(HI):
            nc.tensor.matmul(
                pu[:, r * 128:(r + 1) * 128],
                wselz_sb[:, (r * HI + h) * PHI:(r * HI + h + 1) * PHI],
                fvh[:, h, r, :],
                start=(h == 0), stop=(h == HI - 1))
    usb = mlp.tile([128, BS], BF16)
    nc.vector.tensor_scalar(out=usb[:], in0=pu[:], scalar1=off0sb[:, 0:1],
                            scalar2=None, op0=ALU.add)

    # ---- MLP (bf16, centered) ----
    h1sb = [mlp.tile([128, BS], BF16, tag=f"h1_{k}", name=f"h1sb{k}")
            for k in range(4)]
    h2sb = [mlp.tile([128, BS], BF16, tag=f"h2_{k}", name=f"h2sb{k}")
            for k in range(2)]
    h1t = [mlp.tile([128, BS], F32, tag=f"h1t_{k}", name=f"h1t{k}")
           for k in range(4)]
    h2t = [mlp.tile([128, BS], F32, tag=f"h2t_{k}", name=f"h2t{k}")
           for k in range(2)]

    # W1 and W2 interleaved: W2's kc-partial accumulates as soon as h1[kc]
    # is centered, instead of waiting for all four h1 chunks.
    w2v = w2sb[:].rearrange("p (kc m) -> p kc m", kc=4)
    ph2 = [ps_w2.tile([128, BS], F32, tag=f"ph2_{m}", name=f"ph2_{m}")
           for m in range(2)]
    for kc in range(4):
        ph = ps_mlp.tile([128, BS], F32, tag="ph", name="ph_a")
        nc.tensor.matmul(ph[:], w1sb[:, kc * 128:(kc + 1) * 128], usb[:],
                         start=True, stop=True)
        nc.scalar.activation(h1t[kc][:], ph[:], AF.Relu,
                             bias=b1psb[:, kc:kc + 1], scale=1.0)
        nc.vector.tensor_scalar(out=h1sb[kc][:], in0=h1t[kc][:],
                                scalar1=c1nsb[:, kc:kc + 1], scalar2=None,
                                op0=ALU.add)
        for mc in range(2):
            nc.tensor.matmul(ph2[mc][:],
                             w2v[:, kc, mc * 128:(mc + 1) * 128],
                             h1sb[kc][:], start=(kc == 0), stop=(kc == 3))
    for mc in range(2):
        nc.scalar.activation(h2t[mc][:], ph2[mc][:], AF.Relu,
                             bias=b2psb[:, mc:mc + 1], scale=1.0)
        nc.vector.tensor_scalar(out=h2sb[mc][:], in0=h2t[mc][:],
                                scalar1=c2nsb[:, mc:mc + 1], scalar2=None,
                                op0=ALU.add)
    py = ps_y.tile([1, BS], F32)
    for kc in range(2):
        nc.tensor.matmul(py[:], w3sb[:, kc:kc + 1], h2sb[kc][:],
                         start=(kc == 0), stop=(kc == 1))
    ysb = mlp.tile([1, BS], F32)
    nc.vector.tensor_scalar(out=ysb[:], in0=py[:], scalar1=b3psb[0:1, 0:1],
                            scalar2=None, op0=ALU.add)
    # ysb columns are in (r, k) order; host un-permutes (b = k*4+r)
    nc.sync.dma_start(out.ap(), ysb[:])


_CACHED_NC = None


def _get_nc():
    global _CACHED_NC
    if _CACHED_NC is None:
        _CACHED_NC = build_program()
    return _CACHED_NC


def _prep_in_maps(x, W_phi, b_phi, W1, b1, W2, b2, W3, b3):
    import ml_dtypes
    bf = ml_dtypes.bfloat16

    x = np.asarray(x, dtype=np.int64)
    Wd = np.asarray(W_phi, np.float64)
    bphid = np.asarray(b_phi, np.float64)
    W1d = np.asarray(W1, np.float64)
    W2d = np.asarray(W2, np.float64)
    W3d = np.asarray(W3, np.float64)
    b1d, b2d, b3d = (np.asarray(v, np.float64) for v in (b1, b2, b3))

    Wb = Wd.astype(np.float32).astype(bf).astype(np.float64)
    ubar = (N / C) * Wb.sum(0) + N * bphid
    off0v = (N * bphid - ubar).astype(np.float32)
    b1pv = (b1d + ubar @ W1d).astype(np.float32)
    c1v = np.maximum(b1pv, 0.0)
    b2pv = (b2d + c1v.astype(np.float64) @ W2d).astype(np.float32)
    c2v = np.maximum(b2pv, 0.0)
    b3pv = (b3d + c2v.astype(np.float64) @ W3d).astype(np.float32)

    # wselz[(lo,r) partition, (rv, h, d)] = Wb[32h+lo, d] iff r == rv
    wsel = Wb.astype(np.float32).reshape(HI, LO, PHI)     # [h, lo, d]
    wz = np.zeros((LO, 4, 4, HI, PHI), np.float32)        # [lo, r, rv, h, d]
    for r in range(4):
        wz[:, r, r, :, :] = wsel.transpose(1, 0, 2)
    wselzv = np.ascontiguousarray(
        wz.reshape(128, 4 * HI * PHI).astype(bf))

    w1v = np.ascontiguousarray(W1d.astype(np.float32).astype(bf))
    w2vv = np.ascontiguousarray(
        W2d.astype(np.float32).reshape(4, 128, 2, 128).transpose(1, 0, 2, 3)
        .reshape(128, 1024).astype(bf))
    w3v = np.ascontiguousarray(
        W3d.astype(np.float32).reshape(2, 128).T.astype(bf))

    shared = {
        "iolo": np.ascontiguousarray(np.broadcast_to(
            np.arange(LO, dtype=np.float32), (128, LO)).astype(bf)),
        "iohi": np.ascontiguousarray(np.broadcast_to(
            np.arange(HI, dtype=np.float32), (128, HI)).astype(bf)),
        "wselz": wselzv,
        "w1": w1v,
        "b1p": np.ascontiguousarray(b1pv.reshape(4, 128).T),
        "c1n": np.ascontiguousarray((-c1v).reshape(4, 128).T),
        "w2": w2vv,
        "b2p": np.ascontiguousarray(b2pv.reshape(2, 128).T),
        "c2n": np.ascontiguousarray((-c2v).reshape(2, 128).T),
        "w3": w3v,
        "off0": np.ascontiguousarray(off0v.reshape(128, 1)),
        "b3p": np.ascontiguousarray(b3pv.reshape(1, 1)),
    }

    lo_all = (x & 31).astype(np.float32)
    hi_all = (x >> 5).astype(np.float32)

    def quadpack(v):  # [512b, 512j] -> [128p, concat_blk(i, q_b, r)]
        a = v.T.reshape(NI, 128, 128, 4)             # [i, p, g, r]
        a = a.transpose(1, 0, 2, 3)                  # [p, i, g, r]
        parts = [np.ascontiguousarray(a[:, :, s:s + n, :]).reshape(128, -1)
                 for s, n in zip(QSTART, QS)]
        return np.ascontiguousarray(
            np.concatenate(parts, axis=1).astype(bf))

    maps = []
    for c in range(NCORES):
        sl = slice(c * BS, (c + 1) * BS)
        m = dict(shared)
        m["xlo"] = quadpack(lo_all[sl])
        m["xhi"] = quadpack(hi_all[sl])
        maps.append(m)
    return maps


def run(trace=False, tmpdir=None, **inputs):
    nc = _get_nc()
    in_maps = _prep_in_maps(**inputs)
    res = run_bass_kernel_spmd(nc, in_maps, core_ids=list(range(NCORES)),
                               trace=trace, tmpdir=tmpdir)
    # device emits y in (r, k) order per core; un-permute to b = k*4 + r
    y = np.concatenate([
        np.asarray(res.results[c]["out"]).reshape(4, BS // 4).T.reshape(BS)
        for c in range(NCORES)])
    return y.reshape(B, 1).astype(np.float32), res


def kernel(**inputs):
    y, _ = run(trace=False, **inputs)
    return y

